# revision 38
# baseline (speedup 1.0000x reference)
"""Trainium2 Bass kernel: batched Euler-Maruyama integration of a neural SDE.

Reference computation (per step t):
    vf     = -y + MLP(y)          MLP: tanh(64->256), tanh(256->256) x2, 256->64
    y_next = y + dt_t * vf + SIGMA * sqrt(dt_t) * dW_t
Output: all intermediate states [T+1, B, D].

Strategy
--------
Data-parallel over the particle axis: B=4096 -> 512 particles on each of the
8 NeuronCores.  All math runs in a feature-major ("transposed") layout
Y.T [D, B_local] so every MLP layer is a plain stationary-weight matmul with
the contraction on the partition axis -- no on-device transposes at all:

  * host pre-transposes y0 and the (pre-scaled) noise to [.., D, B_local]
  * PE:  H1 = Win @ Y, H2 = W0 @ H1, H3 = W1 @ H2 (K-chunked, M-folded into
         one PSUM tile per layer), vf_psum = Wout @ H3; fp16 operands
         (1 cycle/row + fast weight load), fp32 PSUM accumulate
  * ACT: tanh epilogues straight out of PSUM (one instr per layer when the
         biases are zero, per-M-tile with per-partition bias otherwise)
  * DVE: v = (1-dt_t) y + noise_t early, then y_next = dt_t vf_psum + v,
         written twice: fp16 shadow (feeds next step's matmuls with minimal
         recurrence latency) and fp32 state (exact accumulator + output)
  * per-step DMA: noise tile in, state tile out (contiguous, pre-transposed)

The batch is split into NCHUNK chunks emitted as a staggered 7-unit software
pipeline so one chunk's matmuls/tanh overlap the other's recurrence tail;
steps are serially dependent so this is the only parallelism axis in a core.
The per-step wall time is bound by the serial chain
tanh1 -> L2 -> tanh2 -> L3 -> tanh3 -> L4 -> DVE -> L1' (~5.2 us measured).

The host prepends y0, transposes outputs back and concatenates the 8 shards.
"""

import numpy as np

B, D, W, T = 4096, 64, 256, 256
NCORES = 8
BL = B // NCORES  # 512 particles per core
SIGMA = 0.1

NCHUNK = 3        # batch chunks per core, staggered for chain-latency hiding
TURBO = False     # layer-1 linearity recursion: measured slower (PE-bound)


def _build(dts, zero_bias, nchunk=NCHUNK, steps=T, bl=BL, turbo=TURBO):
    """Emit the Bass/Tile program. dts: numpy [steps] fp32 per-step dt.

    Matmul operands are fp16 (1 cycle/row on the PE + fast weight load);
    the integration state y stays fp32 (a DVE-produced fp16 shadow feeds
    the matmuls), so state error does not accumulate beyond fp16 MLP noise
    that is scaled by dt each step.
    """
    import concourse.bass as bass  # noqa: F401
    import concourse.mybir as mybir
    import concourse.tile as tile
    from concourse import bacc

    f32 = mybir.dt.float32
    f16 = mybir.dt.float16
    Tanh = mybir.ActivationFunctionType.Tanh
    MULT = mybir.AluOpType.mult
    ADD = mybir.AluOpType.add

    base = bl // nchunk
    rem = bl - base * nchunk
    csizes = [base + (1 if c < rem else 0) for c in range(nchunk)]
    los = [sum(csizes[:c]) for c in range(nchunk)]
    chunks = list(range(nchunk))

    # turbo path: layer-1 pre-activation p(t) = Win @ y(t) is maintained by
    # recursion  p(t+1) = (1-dt) p(t) + dt (Win Wout) H3(t) + Win nz(t)
    # accumulated in PSUM (q = p/dt), removing L4 -> DVE -> L1 from the
    # serial per-step chain.  Needs one (1-dt)/dt-scaled identity per unique
    # dt, so fall back when dts are too diverse.
    uq, uinv = np.unique(np.asarray(dts, np.float32), return_inverse=True)
    U = len(uq)
    turbo = turbo and U <= 8

    nc = bacc.Bacc("TRN2", target_bir_lowering=False, debug=False)

    y0_d = nc.dram_tensor("y0t", [D, bl], f32, kind="ExternalInput")
    y0h_d = nc.dram_tensor("y0th", [D, bl], f16, kind="ExternalInput")
    nz_d = nc.dram_tensor("nz", [steps, D, bl], f32, kind="ExternalInput")
    win_d = nc.dram_tensor("wint", [D, W], f16, kind="ExternalInput")
    wh_d = nc.dram_tensor("wht", [2, 2, 128, W], f16, kind="ExternalInput")
    wout_d = nc.dram_tensor("woutt", [2, 128, D], f16, kind="ExternalInput")
    if turbo:
        nzq_d = nc.dram_tensor("nzq", [steps, D, bl], f16, kind="ExternalInput")
        wfuse_d = nc.dram_tensor("wfuset", [2, 2, 128, 128], f16,
                                 kind="ExternalInput")
        cid_d = nc.dram_tensor("cident", [U, 128, 128], f16,
                               kind="ExternalInput")
    if not zero_bias:
        bias_d = nc.dram_tensor("biases", [3, 128, 2], f32, kind="ExternalInput")
    out_d = nc.dram_tensor("outt", [steps, D, bl], f32, kind="ExternalOutput")

    mm = nc.tensor.matmul

    with tile.TileContext(nc) as tc:
        with (
            tc.tile_pool(name="const", bufs=1) as const,
            tc.tile_pool(name="hbuf", bufs=3) as hbuf,
            tc.tile_pool(name="state", bufs=4) as st,
            tc.tile_pool(name="nzp", bufs=6) as nzp,
            tc.tile_pool(name="psum", bufs=1, space="PSUM") as ps,
        ):
            # ---- constants (loaded once) ----
            win_s = const.tile([D, W], f16)            # W_in.T  [64, 256]
            nc.sync.dma_start(out=win_s[:], in_=win_d[:])
            wh_s = const.tile([128, 2, 2, W], f16)     # hidden lhsT chunks
            for li in range(2):
                for k in range(2):
                    nc.sync.dma_start(out=wh_s[:, li, k, :], in_=wh_d[li, k])
            wout_s = const.tile([128, 2, D], f16)      # W_out.T chunks
            for k in range(2):
                nc.sync.dma_start(out=wout_s[:, k, :], in_=wout_d[k])
            if turbo:
                wfuse_s = const.tile([128, 2, 2, 128], f16)
                for k in range(2):
                    for m in range(2):
                        nc.sync.dma_start(out=wfuse_s[:, k, m, :],
                                          in_=wfuse_d[k, m])
                cid_s = const.tile([128, U, 128], f16)
                for u_ in range(U):
                    nc.sync.dma_start(out=cid_s[:, u_, :], in_=cid_d[u_])
            if not zero_bias:
                bias_s = const.tile([128, 3, 2], f32)
                for j in range(3):
                    nc.sync.dma_start(out=bias_s[:, j, :], in_=bias_d[j])

            # ---- initial state: y (fp32) + fp16 shadow for matmuls ----
            ycur, yhcur = [], []
            for c in chunks:
                csz, lo = csizes[c], los[c]
                y_t = st.tile([D, csz], f32, tag=f"y{c}")
                nc.sync.dma_start(out=y_t[:], in_=y0_d[:, lo:lo + csz])
                yh_t = st.tile([D, csz], f16, tag=f"yh{c}")
                nc.sync.dma_start(out=yh_t[:], in_=y0h_d[:, lo:lo + csz])
                ycur.append(y_t)
                yhcur.append(yh_t)

            def tanh_layer(h_sb, h_ps, li, scale=1.0):
                if zero_bias:
                    # flatten [128, 2, csz] -> [128, 2*csz]: a single free dim
                    # avoids the per-outer-iteration AP restart on ScalarE
                    nc.scalar.activation(
                        out=h_sb.rearrange("p a b -> p (a b)"),
                        in_=h_ps.rearrange("p a b -> p (a b)"),
                        func=Tanh, scale=scale)
                else:
                    for m in range(2):
                        nc.scalar.activation(
                            out=h_sb[:, m, :], in_=h_ps[:, m, :], func=Tanh,
                            bias=bias_s[:, li, m:m + 1], scale=scale)

            # ---- time stepping: 7-unit software pipeline per chunk, with
            #      chunk B lagging ~half a step so its mid-MLP work hides
            #      chunk A's recurrence bubble (L4 -> DVE -> next L1) and
            #      vice versa.  Per-engine program order IS the schedule. ----
            NU = 7
            LAGS = tuple(c * NU // nchunk for c in chunks)
            live = {c: {} for c in chunks}   # per-chunk in-flight tiles

            def new_q(c):
                return ps.tile([128, 2, csizes[c]], f32, tag=f"hAp{c}",
                               name=f"h1p_{c}")

            def unit(c, t, u):
                dt = float(dts[t])
                sc = 1.0 if t == 0 else float(dts[t - 1])  # q -> p scale
                csz, lo = csizes[c], los[c]
                lv = live[c]
                if u == 0:
                    lv['nz'] = nzp.tile([D, csz], f32, tag=f"nz{c}",
                                        name=f"nz_{c}")
                    nc.sync.dma_start(out=lv['nz'][:],
                                      in_=nz_d[t, :, lo:lo + csz])
                    if turbo:
                        lv['nzq'] = nzp.tile([D, csz], f16, tag=f"nzq{c}",
                                             name=f"nzq_{c}")
                        nc.sync.dma_start(out=lv['nzq'][:],
                                          in_=nzq_d[t, :, lo:lo + csz])
                    else:
                        lv['h1p'] = new_q(c)
                        mm(lv['h1p'][:, 0, :], win_s[:, 0:128], yhcur[c][:],
                           start=True, stop=True)
                        mm(lv['h1p'][:, 1, :], win_s[:, 128:256], yhcur[c][:],
                           start=True, stop=True)
                    # v = (1-dt)*y + noise, early: off the serial chain
                    lv['v'] = st.tile([D, csz], f32, tag=f"v{c}", name=f"v_{c}")
                    nc.vector.scalar_tensor_tensor(
                        out=lv['v'][:], in0=ycur[c][:], scalar=1.0 - dt,
                        in1=lv['nz'][:], op0=MULT, op1=ADD)
                elif u == 1:
                    hs = hbuf.tile([128, 2, csz], f16, tag=f"h1{c}",
                                   name=f"h1_{c}")
                    tanh_layer(hs, lv['h1p'], 0, scale=sc if turbo else 1.0)
                    lv['h1'] = hs
                    if turbo and t + 1 < steps:
                        # p(t) in fp16, feeds next step's decay matmul
                        hp16 = st.tile([128, 2, csz], f16, tag=f"hp16{c}",
                                       name=f"hp16_{c}")
                        nc.vector.tensor_scalar_mul(hp16[:], lv['h1p'][:], sc)
                        lv['hp16'] = hp16
                elif u in (3, 5):
                    li = (u - 1) // 2
                    hs = hbuf.tile([128, 2, csz], f16, tag=f"h{li + 1}{c}",
                                   name=f"h{li + 1}_{c}")
                    tanh_layer(hs, lv[f'h{li + 1}p'], li)
                    lv[f'h{li + 1}'] = hs
                elif u in (2, 4):
                    li = (u - 2) // 2
                    hp = ps.tile([128, 2, csz], f32,
                                 tag=(f"hBp{c}" if li == 0 else f"hAp{c}"),
                                 name=f"h{li + 2}p_{c}")
                    hprev = lv[f'h{li + 1}']
                    for m in range(2):
                        for k in range(2):
                            mm(hp[:, m, :],
                               wh_s[:, li, k, m * 128:(m + 1) * 128],
                               hprev[:, k, :], start=(k == 0), stop=(k == 1))
                    lv[f'h{li + 2}p'] = hp
                elif u == 6:
                    if turbo and t + 1 < steps:
                        # accumulate q(t+1) = (1-dt)/dt p(t) + Win nzq(t)
                        #                     + Wfuse H3(t)   (tanh scales by dt)
                        qn = new_q(c)
                        ui = int(uinv[t])
                        for m in range(2):
                            # one accumulation group per bank at a time:
                            # keep each m-region's group contiguous
                            mm(qn[:, m, :], cid_s[:, ui, :],
                               lv['hp16'][:, m, :], start=True, stop=False)
                            mm(qn[:, m, :], win_s[:, m * 128:(m + 1) * 128],
                               lv['nzq'][:], start=False, stop=False)
                            for k in range(2):
                                mm(qn[:, m, :], wfuse_s[:, k, m, :],
                                   lv['h3'][:, k, :], start=False,
                                   stop=(k == 1))
                        lv['h1p'] = qn
                    ypt = ps.tile([D, csz], f32, tag=f"hBp{c}", name=f"yp_{c}")
                    mm(ypt[:], wout_s[:, 0, :], lv['h3'][:, 0, :],
                       start=True, stop=False)
                    mm(ypt[:], wout_s[:, 1, :], lv['h3'][:, 1, :],
                       start=False, stop=True)
                    if not turbo:
                        # fp16 shadow first (feeds next step's matmuls ASAP)
                        yh_nx = st.tile([D, csz], f16, tag=f"yh{c}",
                                        name=f"yh_{c}")
                        nc.vector.scalar_tensor_tensor(
                            out=yh_nx[:], in0=ypt[:], scalar=dt, in1=lv['v'][:],
                            op0=MULT, op1=ADD)
                        yhcur[c] = yh_nx
                    y_nx = st.tile([D, csz], f32, tag=f"y{c}", name=f"y_{c}")
                    nc.vector.scalar_tensor_tensor(
                        out=y_nx[:], in0=ypt[:], scalar=dt, in1=lv['v'][:],
                        op0=MULT, op1=ADD)
                    nc.sync.dma_start(out=out_d[t, :, lo:lo + csz], in_=y_nx[:])
                    ycur[c] = y_nx

            if turbo:
                # q(0) = Win @ y0 (tanh_layer uses scale 1.0 at t=0)
                for c in chunks:
                    q0 = new_q(c)
                    mm(q0[:, 0, :], win_s[:, 0:128], yhcur[c][:],
                       start=True, stop=True)
                    mm(q0[:, 1, :], win_s[:, 128:256], yhcur[c][:],
                       start=True, stop=True)
                    live[c]['h1p'] = q0

            total = steps * NU + max(LAGS)
            for g in range(total):
                for c in chunks:
                    gg = g - LAGS[c]
                    if 0 <= gg < steps * NU:
                        t, u = divmod(gg, NU)
                        unit(c, t, u)
    nc.compile()
    return nc


def _host_prep(ts, y0, dW, w_in, b_in, w_h, b_h, w_out, b_out):
    """Compute per-core input maps + build params. Returns (in_maps, dts, zero_bias)."""
    f = np.float32
    ts = np.asarray(ts, f)
    dts = (ts[1:] - ts[:-1]).astype(f)
    assert dts.shape[0] == T

    zero_bias = (not np.any(b_in)) and (not np.any(b_h))

    # noise folded with its scale (and b_out drift term): sigma*sqrt(dt)*dW + dt*b_out
    scale = (SIGMA * np.sqrt(dts)).astype(f)  # [T]
    drift = (dts[:, None] * np.asarray(b_out, f)[None, :]).astype(f)  # [T, D]

    h = np.float16
    w_inT = np.ascontiguousarray(np.asarray(w_in, f).T.astype(h))    # [64, 256]
    whT = np.ascontiguousarray(
        np.stack([np.asarray(w_h[i], f).T.reshape(2, 128, W) for i in range(2)])
    ).astype(h)                                                      # [2, 2, 128, 256]
    w_outT = np.ascontiguousarray(np.asarray(w_out, f).T.reshape(2, 128, D)).astype(h)

    uq = np.unique(dts)
    turbo = TURBO and len(uq) <= 8
    if turbo:
        wfuse = (np.asarray(w_in, f) @ np.asarray(w_out, f))         # [256, 256]
        wfuseT = np.ascontiguousarray(
            wfuse.T.reshape(2, 128, 2, 128).transpose(0, 2, 1, 3)).astype(h)
        cident = np.stack([((1.0 - u) / u) * np.eye(128, dtype=f)
                           for u in uq]).astype(h)                   # [U, 128, 128]
        nzq_scale = (SIGMA / np.sqrt(dts)).astype(f)                 # [T]
    biases = np.zeros((3, 128, 2), f)
    biases[0] = np.asarray(b_in, f).reshape(2, 128).T
    biases[1] = np.asarray(b_h[0], f).reshape(2, 128).T
    biases[2] = np.asarray(b_h[1], f).reshape(2, 128).T

    y0 = np.asarray(y0, f)
    dW = np.asarray(dW, f)

    in_maps = []
    for c in range(NCORES):
        lo = c * BL
        nzc = dW[:, lo:lo + BL, :] * scale[:, None, None] + drift[:, None, :]
        nzc = np.ascontiguousarray(nzc.transpose(0, 2, 1)).astype(f)  # [T, 64, BL]
        y0tc = np.ascontiguousarray(y0[lo:lo + BL].T)
        m = {
            "y0t": y0tc,
            "y0th": y0tc.astype(np.float16),
            "nz": nzc,
            "wint": w_inT,
            "wht": whT,
            "woutt": w_outT,
        }
        if turbo:
            nzqc = dW[:, lo:lo + BL, :] * nzq_scale[:, None, None] \
                + np.asarray(b_out, f)[None, None, :]
            m["nzq"] = np.ascontiguousarray(
                nzqc.transpose(0, 2, 1)).astype(np.float16)
            m["wfuset"] = wfuseT
            m["cident"] = cident
        if not zero_bias:
            m["biases"] = biases
        in_maps.append(m)
    return in_maps, dts, zero_bias


_NC_CACHE = {}

# test-harness hooks (kernel() ignores these unless set by test code)
TRACE = False
LAST_RESULT = None


def kernel(ts, y0, dW, w_in, b_in, w_h, b_h, w_out, b_out):
    global LAST_RESULT
    from concourse.bass_utils import run_bass_kernel_spmd

    in_maps, dts, zero_bias = _host_prep(
        ts, y0, dW, w_in, b_in, w_h, b_h, w_out, b_out)

    key = (zero_bias, np.asarray(dts).tobytes())
    nc = _NC_CACHE.get(key)
    if nc is None:
        nc = _build(dts, zero_bias)
        _NC_CACHE[key] = nc

    res = run_bass_kernel_spmd(nc, in_maps, core_ids=list(range(NCORES)),
                               trace=TRACE)
    LAST_RESULT = res

    out = np.empty((T + 1, B, D), np.float32)
    out[0] = np.asarray(y0, np.float32)
    for c in range(NCORES):
        lo = c * BL
        out[1:, lo:lo + BL, :] = res.results[c]["outt"].transpose(0, 2, 1)
    return out


# revision 39
# speedup vs baseline: 1.0015x; 1.0015x over previous
"""Trainium2 Bass kernel: batched Euler-Maruyama integration of a neural SDE.

Reference computation (per step t):
    vf     = -y + MLP(y)          MLP: tanh(64->256), tanh(256->256) x2, 256->64
    y_next = y + dt_t * vf + SIGMA * sqrt(dt_t) * dW_t
Output: all intermediate states [T+1, B, D].

Strategy
--------
Data-parallel over the particle axis: B=4096 -> 512 particles on each of the
8 NeuronCores.  All math runs in a feature-major ("transposed") layout
Y.T [D, B_local] so every MLP layer is a plain stationary-weight matmul with
the contraction on the partition axis -- no on-device transposes at all:

  * host pre-transposes y0 and the (pre-scaled) noise to [.., D, B_local]
  * PE:  H1 = Win @ Y, H2 = W0 @ H1, H3 = W1 @ H2 (K-chunked, M-folded into
         one PSUM tile per layer), vf_psum = Wout @ H3; fp16 operands
         (1 cycle/row + fast weight load), fp32 PSUM accumulate
  * ACT: tanh epilogues straight out of PSUM (one instr per layer when the
         biases are zero, per-M-tile with per-partition bias otherwise)
  * DVE: v = (1-dt_t) y + noise_t early, then y_next = dt_t vf_psum + v,
         written twice: fp16 shadow (feeds next step's matmuls with minimal
         recurrence latency) and fp32 state (exact accumulator + output)
  * per-step DMA: noise tile in, state tile out (contiguous, pre-transposed)

The batch is split into NCHUNK=3 chunks emitted as a staggered 7-unit
software pipeline so each chunk's serial recurrence chain (tanh1 -> L2 ->
tanh2 -> L3 -> tanh3 -> L4 -> DVE -> L1') is hidden behind the other chunks'
work; steps are serially dependent so this is the only parallelism axis in a
core.  Smaller chunks shorten that chain; 3 is the optimum before ScalarE
per-instruction overhead dominates (measured ~4.7 us/step, ACT-bound).
PSUM fits because layer-1/layer-3 tiles share one bank per chunk and
layer-2/output share another (their lifetimes never overlap in the pipeline).

The host prepends y0, transposes outputs back and concatenates the 8 shards.
"""

import numpy as np

B, D, W, T = 4096, 64, 256, 256
NCORES = 8
BL = B // NCORES  # 512 particles per core
SIGMA = 0.1

NCHUNK = 3        # batch chunks per core, staggered for chain-latency hiding
TURBO = False     # layer-1 linearity recursion: measured slower (PE-bound)


def _build(dts, zero_bias, nchunk=NCHUNK, steps=T, bl=BL, turbo=TURBO):
    """Emit the Bass/Tile program. dts: numpy [steps] fp32 per-step dt.

    Matmul operands are fp16 (1 cycle/row on the PE + fast weight load);
    the integration state y stays fp32 (a DVE-produced fp16 shadow feeds
    the matmuls), so state error does not accumulate beyond fp16 MLP noise
    that is scaled by dt each step.
    """
    import concourse.bass as bass  # noqa: F401
    import concourse.mybir as mybir
    import concourse.tile as tile
    from concourse import bacc

    f32 = mybir.dt.float32
    f16 = mybir.dt.float16
    Tanh = mybir.ActivationFunctionType.Tanh
    MULT = mybir.AluOpType.mult
    ADD = mybir.AluOpType.add

    base = bl // nchunk
    rem = bl - base * nchunk
    csizes = [base + (1 if c < rem else 0) for c in range(nchunk)]
    los = [sum(csizes[:c]) for c in range(nchunk)]
    chunks = list(range(nchunk))

    # turbo path: layer-1 pre-activation p(t) = Win @ y(t) is maintained by
    # recursion  p(t+1) = (1-dt) p(t) + dt (Win Wout) H3(t) + Win nz(t)
    # accumulated in PSUM (q = p/dt), removing L4 -> DVE -> L1 from the
    # serial per-step chain.  Needs one (1-dt)/dt-scaled identity per unique
    # dt, so fall back when dts are too diverse.
    uq, uinv = np.unique(np.asarray(dts, np.float32), return_inverse=True)
    U = len(uq)
    turbo = turbo and U <= 8

    nc = bacc.Bacc("TRN2", target_bir_lowering=False, debug=False)

    y0_d = nc.dram_tensor("y0t", [D, bl], f32, kind="ExternalInput")
    y0h_d = nc.dram_tensor("y0th", [D, bl], f16, kind="ExternalInput")
    nz_d = nc.dram_tensor("nz", [steps, D, bl], f32, kind="ExternalInput")
    win_d = nc.dram_tensor("wint", [D, W], f16, kind="ExternalInput")
    wh_d = nc.dram_tensor("wht", [2, 2, 128, W], f16, kind="ExternalInput")
    wout_d = nc.dram_tensor("woutt", [2, 128, D], f16, kind="ExternalInput")
    if turbo:
        nzq_d = nc.dram_tensor("nzq", [steps, D, bl], f16, kind="ExternalInput")
        wfuse_d = nc.dram_tensor("wfuset", [2, 2, 128, 128], f16,
                                 kind="ExternalInput")
        cid_d = nc.dram_tensor("cident", [U, 128, 128], f16,
                               kind="ExternalInput")
    if not zero_bias:
        bias_d = nc.dram_tensor("biases", [3, 128, 2], f32, kind="ExternalInput")
    out_d = nc.dram_tensor("outt", [steps, D, bl], f32, kind="ExternalOutput")

    mm = nc.tensor.matmul

    with tile.TileContext(nc) as tc:
        with (
            tc.tile_pool(name="const", bufs=1) as const,
            tc.tile_pool(name="hbuf", bufs=3) as hbuf,
            tc.tile_pool(name="state", bufs=4) as st,
            tc.tile_pool(name="nzp", bufs=6) as nzp,
            tc.tile_pool(name="psum", bufs=1, space="PSUM") as ps,
        ):
            # ---- constants (loaded once) ----
            win_s = const.tile([D, W], f16)            # W_in.T  [64, 256]
            nc.sync.dma_start(out=win_s[:], in_=win_d[:])
            wh_s = const.tile([128, 2, 2, W], f16)     # hidden lhsT chunks
            for li in range(2):
                for k in range(2):
                    nc.sync.dma_start(out=wh_s[:, li, k, :], in_=wh_d[li, k])
            wout_s = const.tile([128, 2, D], f16)      # W_out.T chunks
            for k in range(2):
                nc.sync.dma_start(out=wout_s[:, k, :], in_=wout_d[k])
            if turbo:
                wfuse_s = const.tile([128, 2, 2, 128], f16)
                for k in range(2):
                    for m in range(2):
                        nc.sync.dma_start(out=wfuse_s[:, k, m, :],
                                          in_=wfuse_d[k, m])
                cid_s = const.tile([128, U, 128], f16)
                for u_ in range(U):
                    nc.sync.dma_start(out=cid_s[:, u_, :], in_=cid_d[u_])
            if not zero_bias:
                bias_s = const.tile([128, 3, 2], f32)
                for j in range(3):
                    nc.sync.dma_start(out=bias_s[:, j, :], in_=bias_d[j])

            # ---- initial state: y (fp32) + fp16 shadow for matmuls ----
            ycur, yhcur = [], []
            for c in chunks:
                csz, lo = csizes[c], los[c]
                y_t = st.tile([D, csz], f32, tag=f"y{c}")
                nc.sync.dma_start(out=y_t[:], in_=y0_d[:, lo:lo + csz])
                yh_t = st.tile([D, csz], f16, tag=f"yh{c}")
                nc.sync.dma_start(out=yh_t[:], in_=y0h_d[:, lo:lo + csz])
                ycur.append(y_t)
                yhcur.append(yh_t)

            def tanh_layer(h_sb, h_ps, li, scale=1.0):
                if zero_bias:
                    # flatten [128, 2, csz] -> [128, 2*csz]: a single free dim
                    # avoids the per-outer-iteration AP restart on ScalarE
                    nc.scalar.activation(
                        out=h_sb.rearrange("p a b -> p (a b)"),
                        in_=h_ps.rearrange("p a b -> p (a b)"),
                        func=Tanh, scale=scale)
                else:
                    for m in range(2):
                        nc.scalar.activation(
                            out=h_sb[:, m, :], in_=h_ps[:, m, :], func=Tanh,
                            bias=bias_s[:, li, m:m + 1], scale=scale)

            # ---- time stepping: 7-unit software pipeline per chunk, with
            #      chunk B lagging ~half a step so its mid-MLP work hides
            #      chunk A's recurrence bubble (L4 -> DVE -> next L1) and
            #      vice versa.  Per-engine program order IS the schedule. ----
            NU = 7
            LAGS = tuple(c * NU // nchunk for c in chunks)
            live = {c: {} for c in chunks}   # per-chunk in-flight tiles

            def new_q(c):
                return ps.tile([128, 2, csizes[c]], f32, tag=f"hAp{c}",
                               name=f"h1p_{c}")

            def unit(c, t, u):
                dt = float(dts[t])
                sc = 1.0 if t == 0 else float(dts[t - 1])  # q -> p scale
                csz, lo = csizes[c], los[c]
                lv = live[c]
                if u == 0:
                    lv['nz'] = nzp.tile([D, csz], f32, tag=f"nz{c}",
                                        name=f"nz_{c}")
                    nc.sync.dma_start(out=lv['nz'][:],
                                      in_=nz_d[t, :, lo:lo + csz])
                    if turbo:
                        lv['nzq'] = nzp.tile([D, csz], f16, tag=f"nzq{c}",
                                             name=f"nzq_{c}")
                        nc.sync.dma_start(out=lv['nzq'][:],
                                          in_=nzq_d[t, :, lo:lo + csz])
                    else:
                        lv['h1p'] = new_q(c)
                        mm(lv['h1p'][:, 0, :], win_s[:, 0:128], yhcur[c][:],
                           start=True, stop=True)
                        mm(lv['h1p'][:, 1, :], win_s[:, 128:256], yhcur[c][:],
                           start=True, stop=True)
                    # v = (1-dt)*y + noise, early: off the serial chain
                    lv['v'] = st.tile([D, csz], f32, tag=f"v{c}", name=f"v_{c}")
                    nc.vector.scalar_tensor_tensor(
                        out=lv['v'][:], in0=ycur[c][:], scalar=1.0 - dt,
                        in1=lv['nz'][:], op0=MULT, op1=ADD)
                elif u == 1:
                    hs = hbuf.tile([128, 2, csz], f16, tag=f"h1{c}",
                                   name=f"h1_{c}")
                    tanh_layer(hs, lv['h1p'], 0, scale=sc if turbo else 1.0)
                    lv['h1'] = hs
                    if turbo and t + 1 < steps:
                        # p(t) in fp16, feeds next step's decay matmul
                        hp16 = st.tile([128, 2, csz], f16, tag=f"hp16{c}",
                                       name=f"hp16_{c}")
                        nc.vector.tensor_scalar_mul(hp16[:], lv['h1p'][:], sc)
                        lv['hp16'] = hp16
                elif u in (3, 5):
                    li = (u - 1) // 2
                    hs = hbuf.tile([128, 2, csz], f16, tag=f"h{li + 1}{c}",
                                   name=f"h{li + 1}_{c}")
                    tanh_layer(hs, lv[f'h{li + 1}p'], li)
                    lv[f'h{li + 1}'] = hs
                elif u in (2, 4):
                    li = (u - 2) // 2
                    hp = ps.tile([128, 2, csz], f32,
                                 tag=(f"hBp{c}" if li == 0 else f"hAp{c}"),
                                 name=f"h{li + 2}p_{c}")
                    hprev = lv[f'h{li + 1}']
                    for m in range(2):
                        for k in range(2):
                            mm(hp[:, m, :],
                               wh_s[:, li, k, m * 128:(m + 1) * 128],
                               hprev[:, k, :], start=(k == 0), stop=(k == 1))
                    lv[f'h{li + 2}p'] = hp
                elif u == 6:
                    if turbo and t + 1 < steps:
                        # accumulate q(t+1) = (1-dt)/dt p(t) + Win nzq(t)
                        #                     + Wfuse H3(t)   (tanh scales by dt)
                        qn = new_q(c)
                        ui = int(uinv[t])
                        for m in range(2):
                            # one accumulation group per bank at a time:
                            # keep each m-region's group contiguous
                            mm(qn[:, m, :], cid_s[:, ui, :],
                               lv['hp16'][:, m, :], start=True, stop=False)
                            mm(qn[:, m, :], win_s[:, m * 128:(m + 1) * 128],
                               lv['nzq'][:], start=False, stop=False)
                            for k in range(2):
                                mm(qn[:, m, :], wfuse_s[:, k, m, :],
                                   lv['h3'][:, k, :], start=False,
                                   stop=(k == 1))
                        lv['h1p'] = qn
                    ypt = ps.tile([D, csz], f32, tag=f"hBp{c}", name=f"yp_{c}")
                    mm(ypt[:], wout_s[:, 0, :], lv['h3'][:, 0, :],
                       start=True, stop=False)
                    mm(ypt[:], wout_s[:, 1, :], lv['h3'][:, 1, :],
                       start=False, stop=True)
                    if not turbo:
                        # fp16 shadow first (feeds next step's matmuls ASAP)
                        yh_nx = st.tile([D, csz], f16, tag=f"yh{c}",
                                        name=f"yh_{c}")
                        nc.vector.scalar_tensor_tensor(
                            out=yh_nx[:], in0=ypt[:], scalar=dt, in1=lv['v'][:],
                            op0=MULT, op1=ADD)
                        yhcur[c] = yh_nx
                    y_nx = st.tile([D, csz], f32, tag=f"y{c}", name=f"y_{c}")
                    nc.vector.scalar_tensor_tensor(
                        out=y_nx[:], in0=ypt[:], scalar=dt, in1=lv['v'][:],
                        op0=MULT, op1=ADD)
                    nc.sync.dma_start(out=out_d[t, :, lo:lo + csz], in_=y_nx[:])
                    ycur[c] = y_nx

            if turbo:
                # q(0) = Win @ y0 (tanh_layer uses scale 1.0 at t=0)
                for c in chunks:
                    q0 = new_q(c)
                    mm(q0[:, 0, :], win_s[:, 0:128], yhcur[c][:],
                       start=True, stop=True)
                    mm(q0[:, 1, :], win_s[:, 128:256], yhcur[c][:],
                       start=True, stop=True)
                    live[c]['h1p'] = q0

            total = steps * NU + max(LAGS)
            for g in range(total):
                for c in chunks:
                    gg = g - LAGS[c]
                    if 0 <= gg < steps * NU:
                        t, u = divmod(gg, NU)
                        unit(c, t, u)
    nc.compile()
    return nc


def _host_prep(ts, y0, dW, w_in, b_in, w_h, b_h, w_out, b_out):
    """Compute per-core input maps + build params. Returns (in_maps, dts, zero_bias)."""
    f = np.float32
    ts = np.asarray(ts, f)
    dts = (ts[1:] - ts[:-1]).astype(f)
    assert dts.shape[0] == T

    zero_bias = (not np.any(b_in)) and (not np.any(b_h))

    # noise folded with its scale (and b_out drift term): sigma*sqrt(dt)*dW + dt*b_out
    scale = (SIGMA * np.sqrt(dts)).astype(f)  # [T]
    drift = (dts[:, None] * np.asarray(b_out, f)[None, :]).astype(f)  # [T, D]

    h = np.float16
    w_inT = np.ascontiguousarray(np.asarray(w_in, f).T.astype(h))    # [64, 256]
    whT = np.ascontiguousarray(
        np.stack([np.asarray(w_h[i], f).T.reshape(2, 128, W) for i in range(2)])
    ).astype(h)                                                      # [2, 2, 128, 256]
    w_outT = np.ascontiguousarray(np.asarray(w_out, f).T.reshape(2, 128, D)).astype(h)

    uq = np.unique(dts)
    turbo = TURBO and len(uq) <= 8
    if turbo:
        wfuse = (np.asarray(w_in, f) @ np.asarray(w_out, f))         # [256, 256]
        wfuseT = np.ascontiguousarray(
            wfuse.T.reshape(2, 128, 2, 128).transpose(0, 2, 1, 3)).astype(h)
        cident = np.stack([((1.0 - u) / u) * np.eye(128, dtype=f)
                           for u in uq]).astype(h)                   # [U, 128, 128]
        nzq_scale = (SIGMA / np.sqrt(dts)).astype(f)                 # [T]
    biases = np.zeros((3, 128, 2), f)
    biases[0] = np.asarray(b_in, f).reshape(2, 128).T
    biases[1] = np.asarray(b_h[0], f).reshape(2, 128).T
    biases[2] = np.asarray(b_h[1], f).reshape(2, 128).T

    y0 = np.asarray(y0, f)
    dW = np.asarray(dW, f)

    in_maps = []
    for c in range(NCORES):
        lo = c * BL
        nzc = dW[:, lo:lo + BL, :] * scale[:, None, None] + drift[:, None, :]
        nzc = np.ascontiguousarray(nzc.transpose(0, 2, 1)).astype(f)  # [T, 64, BL]
        y0tc = np.ascontiguousarray(y0[lo:lo + BL].T)
        m = {
            "y0t": y0tc,
            "y0th": y0tc.astype(np.float16),
            "nz": nzc,
            "wint": w_inT,
            "wht": whT,
            "woutt": w_outT,
        }
        if turbo:
            nzqc = dW[:, lo:lo + BL, :] * nzq_scale[:, None, None] \
                + np.asarray(b_out, f)[None, None, :]
            m["nzq"] = np.ascontiguousarray(
                nzqc.transpose(0, 2, 1)).astype(np.float16)
            m["wfuset"] = wfuseT
            m["cident"] = cident
        if not zero_bias:
            m["biases"] = biases
        in_maps.append(m)
    return in_maps, dts, zero_bias


_NC_CACHE = {}

# test-harness hooks (kernel() ignores these unless set by test code)
TRACE = False
LAST_RESULT = None


def kernel(ts, y0, dW, w_in, b_in, w_h, b_h, w_out, b_out):
    global LAST_RESULT
    from concourse.bass_utils import run_bass_kernel_spmd

    in_maps, dts, zero_bias = _host_prep(
        ts, y0, dW, w_in, b_in, w_h, b_h, w_out, b_out)

    key = (zero_bias, np.asarray(dts).tobytes())
    nc = _NC_CACHE.get(key)
    if nc is None:
        nc = _build(dts, zero_bias)
        _NC_CACHE[key] = nc

    res = run_bass_kernel_spmd(nc, in_maps, core_ids=list(range(NCORES)),
                               trace=TRACE)
    LAST_RESULT = res

    out = np.empty((T + 1, B, D), np.float32)
    out[0] = np.asarray(y0, np.float32)
    for c in range(NCORES):
        lo = c * BL
        out[1:, lo:lo + BL, :] = res.results[c]["outt"].transpose(0, 2, 1)
    return out
